# revision 23
# baseline (speedup 1.0000x reference)
"""Trainium2 Bass kernel: sparse-attention transformer block.

Reference computation (N=4096, D=256, H=8, DH=32):
    h  = LN(x; g1, b1)
    q, k, v = (h@Wq+bq, h@Wk+bk, h@Wv+bv) split into 8 heads of 32
    att = softmax over edge-masked q k^T / sqrt(32)   (mask from edge_index)
    x  = x + att@v @ Wo + bo
    x  = x + gelu(LN(x; g2, b2) @ Wm1 + bm1) @ Wm2 + bm2

Sharding: rows are split 512 per core across 8 NeuronCores. Every matmul,
softmax row, layernorm row and residual is row-local, so there are no
collectives. Each core redundantly computes k/v (and LN1 stats) for all 4096
rows. LN gains/shifts are folded into the weight matrices on the host
(Wq <- diag(g1) Wq etc.), so the device only materializes the pure
normalization x_hat = (x - mean) * rsqrt(var + eps).

All transposes the PE needs are produced either by computing W^T @ x_hat^T
directly or via one DRAM-bounce DMA transpose (bf16).
"""

import math

import numpy as np
import ml_dtypes

import concourse.bass as bass
import concourse.bacc as bacc
import concourse.tile as tile
from concourse import mybir
from concourse.bass_utils import run_bass_kernel_spmd

N = 4096
D = 256
H = 8
DH = 32
NCORES = 8
RPC = N // NCORES  # rows per core = 512
P = 128
EPS = 1e-5
BF16 = mybir.dt.bfloat16
F32 = mybir.dt.float32

_CACHE = {}


def _build(has_qkv_bias, has_o_bias, has_m1_bias, has_m2_bias):
    nc = bacc.Bacc("TRN2", target_bir_lowering=False, debug=False)
    AF = mybir.ActivationFunctionType
    OP = mybir.AluOpType

    # ---------------- DRAM I/O ----------------
    x_full_d = nc.dram_tensor("x_full", [N, D], F32, kind="ExternalInput")
    x_own_d = nc.dram_tensor("x_own", [RPC, D], F32, kind="ExternalInput")
    maskT_d = nc.dram_tensor("maskT", [N, RPC], BF16, kind="ExternalInput")
    wkv_d = nc.dram_tensor("wkv", [2, P, 2 * D], BF16, kind="ExternalInput")
    wq_d = nc.dram_tensor("wq", [2, P, D], BF16, kind="ExternalInput")
    wo_d = nc.dram_tensor("wo", [2, P, D], BF16, kind="ExternalInput")
    wm1_d = nc.dram_tensor("wm1", [2, P, 2 * D], BF16, kind="ExternalInput")
    wm2_d = nc.dram_tensor("wm2", [4, P, D], BF16, kind="ExternalInput")
    # biases packed [kv(512) | q(256) | o(256) | m1(512) | m2(256)] = 1792
    bias_d = nc.dram_tensor("bias", [1, 1792], BF16, kind="ExternalInput")
    out_d = nc.dram_tensor("out", [RPC, D], F32, kind="ExternalOutput")

    NT = N // P  # 32 full-row tiles
    OT = RPC // P  # 4 own-row tiles

    from contextlib import ExitStack
    with tile.TileContext(nc) as tc, ExitStack() as es:
        dram = es.enter_context(tc.tile_pool(name="dram", bufs=1, space="DRAM"))
        persist = es.enter_context(tc.tile_pool(name="persist", bufs=1))
        xpool = es.enter_context(tc.tile_pool(name="xpool", bufs=6))
        spool = es.enter_context(tc.tile_pool(name="spool", bufs=8))
        mpool = es.enter_context(tc.tile_pool(name="mpool", bufs=8))
        apool = es.enter_context(tc.tile_pool(name="apool", bufs=10))
        pp_big = es.enter_context(tc.tile_pool(name="pp_big", bufs=2, space="PSUM"))
        pp_y = es.enter_context(tc.tile_pool(name="pp_y", bufs=1, space="PSUM"))

        # DRAM bounce buffers
        xhat_b = dram.tile([N, D], BF16)
        xhato_b = dram.tile([RPC, D], BF16)
        y_b = dram.tile([RPC, D], BF16)
        x2h_b = dram.tile([RPC, D], BF16)
        m1_b = dram.tile([RPC, 2 * D], BF16)

        # persistent SBUF
        xhat_sb = persist.tile([P, NT, D], BF16)
        xhato_sb = persist.tile([P, OT, D], BF16)
        x_own_sb = persist.tile([P, OT, D], F32)
        xhatT = [persist.tile([P, N], BF16, name=f"xhatT{i}") for i in range(2)]
        xhatoT = persist.tile([P, 2, RPC], BF16)
        # k^T stored as head-pair tensors [64, N]; q^T per head zero-padded to
        # [64, RPC] so the scores matmul contracts 64 partitions from base 0.
        kT = [persist.tile([2 * DH, N], BF16, name=f"kT{i}") for i in range(4)]
        qT_pad = [persist.tile([2 * DH, RPC], BF16, name=f"qT{h}") for h in range(H)]
        v_aug = persist.tile([P, NT, H, DH + 1], BF16)
        y_sb = persist.tile([P, OT, D], BF16)
        x2_sb = persist.tile([P, OT, D], F32)
        x2h_sb = persist.tile([P, OT, D], BF16)
        m1_sb = persist.tile([P, OT, 2 * D], BF16)
        m1T = persist.tile([P, 4, RPC], BF16)
        out_sb = persist.tile([P, OT, D], F32)
        wkv_sb = persist.tile([P, 2, 2 * D], BF16)
        wq_sb = persist.tile([P, 2, D], BF16)
        wo_sb = persist.tile([P, 2, D], BF16)
        wm1_sb = persist.tile([P, 2, 2 * D], BF16)
        wm2_sb = persist.tile([P, 4, D], BF16)
        bias_sb = persist.tile([1, 1792], BF16)
        ones_sb = persist.tile([P, 1], BF16)
        onesrow_sb = persist.tile([1, RPC], BF16)
        eps_sb = persist.tile([P, 1], F32)

        nc.vector.memset(ones_sb[:], 1.0)
        nc.vector.memset(onesrow_sb[:], 1.0)
        nc.vector.memset(eps_sb[:], EPS)

        # weights in
        nc.sync.dma_start(out=wkv_sb[:], in_=wkv_d.rearrange("c p f -> p c f"))
        nc.sync.dma_start(out=wq_sb[:], in_=wq_d.rearrange("c p f -> p c f"))
        nc.sync.dma_start(out=wo_sb[:], in_=wo_d.rearrange("c p f -> p c f"))
        nc.sync.dma_start(out=wm1_sb[:], in_=wm1_d.rearrange("c p f -> p c f"))
        nc.sync.dma_start(out=wm2_sb[:], in_=wm2_d.rearrange("c p f -> p c f"))
        nc.sync.dma_start(out=bias_sb[:], in_=bias_d[:])
        b_kv = bias_sb[:, 0:512]
        b_q = bias_sb[:, 512:768]
        b_o = bias_sb[:, 768:1024]
        b_m1 = bias_sb[:, 1024:1536]
        b_m2 = bias_sb[:, 1536:1792]

        # ---------- Phase 1: LN1 stats + x_hat (full, and own rows) ----------
        def ln_tiles(src_ap_fn, ntiles, dst_sb, keep_x=None):
            for i in range(ntiles):
                xt = xpool.tile([P, D], F32, tag="xt")
                nc.sync.dma_start(out=xt[:], in_=src_ap_fn(i))
                if keep_x is not None:
                    nc.vector.tensor_copy(keep_x[:, i, :], xt[:])
                st = spool.tile([P, 6], F32, tag="st")
                nc.vector.bn_stats(out=st[:], in_=xt[:])
                mv = spool.tile([P, 2], F32, tag="mv")
                nc.vector.bn_aggr(out=mv[:], in_=st[:])
                sd = spool.tile([P, 1], F32, tag="sd")
                nc.scalar.activation(
                    out=sd[:], in_=mv[:, 1:2], func=AF.Sqrt, bias=eps_sb[:], scale=1.0
                )
                s = spool.tile([P, 1], F32, tag="s")
                nc.vector.reciprocal(out=s[:], in_=sd[:])
                t = spool.tile([P, 1], F32, tag="t")
                nc.vector.scalar_tensor_tensor(
                    out=t[:], in0=mv[:, 0:1], scalar=-1.0, in1=s[:],
                    op0=OP.mult, op1=OP.mult,
                )
                nc.vector.tensor_scalar(
                    out=dst_sb[:, i, :], in0=xt[:], scalar1=s[:], scalar2=t[:],
                    op0=OP.mult, op1=OP.add,
                )

        ln_tiles(lambda i: x_full_d[i * P:(i + 1) * P, :], NT, xhat_sb)
        ln_tiles(lambda i: x_own_d[i * P:(i + 1) * P, :], OT, xhato_sb,
                 keep_x=x_own_sb)

        # ---------- Phase 2: bounce + transpose x_hat ----------
        nc.sync.dma_start(
            out=xhat_b.rearrange("(t p) d -> p t d", p=P), in_=xhat_sb[:]
        )
        nc.sync.dma_start(
            out=xhato_b.rearrange("(t p) d -> p t d", p=P), in_=xhato_sb[:]
        )
        for half in range(2):
            nc.sync.dma_start_transpose(
                xhatT[half][:], xhat_b[:, half * P:(half + 1) * P]
            )
            nc.sync.dma_start_transpose(
                xhatoT[:, half, :], xhato_b[:, half * P:(half + 1) * P]
            )

        # ---------- Phase 3: projections kT, v, qT ----------
        # kT[pair][do, c] = sum_di Wk[di, 64*pair + do] xhatT[di, c]
        G = 8  # 512-wide column groups of kT, two per PSUM tile
        for pair in range(4):
            for g2 in range(G // 2):
                ps = pp_big.tile([2 * DH, 2, 512], F32, tag="ps")
                for sub in range(2):
                    g = 2 * g2 + sub
                    cs = slice(g * 512, (g + 1) * 512)
                    for ch in range(2):
                        nc.tensor.matmul(
                            ps[:, sub, :],
                            wkv_sb[:, ch, pair * 64:(pair + 1) * 64],
                            xhatT[ch][:, cs],
                            start=(ch == 0),
                            stop=(ch == 1) and not has_qkv_bias,
                        )
                    if has_qkv_bias:
                        # += bk[do] * ones[c]
                        nc.tensor.matmul(
                            ps[:, sub, :],
                            b_kv[:, pair * 64:(pair + 1) * 64],
                            onesrow_sb[:, 0:512],
                            start=False, stop=True,
                        )
                nc.any.tensor_copy(
                    kT[pair][:, g2 * 1024:(g2 + 1) * 1024],
                    ps[:].rearrange("p a b -> p (a b)"),
                )
        # v[r, dv] tiles, copied into v_aug [P, j, h, 33]: 32 head dims plus a
        # constant-1 column so att@v and the softmax denominator share matmuls
        nc.vector.memset(v_aug[:], 1.0)
        for i2 in range(NT // 2):
            ps = pp_big.tile([P, 2, D], F32, tag="ps")
            for sub in range(2):
                i = 2 * i2 + sub
                for ch in range(2):
                    nc.tensor.matmul(
                        ps[:, sub, :],
                        xhatT[ch][:, i * P:(i + 1) * P],
                        wkv_sb[:, ch, D:2 * D],
                        start=(ch == 0),
                        stop=(ch == 1) and not has_qkv_bias,
                    )
                if has_qkv_bias:
                    nc.tensor.matmul(
                        ps[:, sub, :], onesrow_sb[:, 0:P], b_kv[:, D:2 * D],
                        start=False, stop=True,
                    )
            nc.any.tensor_copy(
                v_aug[:, 2 * i2:2 * i2 + 2, :, 0:DH],
                ps[:].rearrange("p s (h d) -> p s h d", h=H),
            )
        # qT_pad[h][do, r own]: head h occupies rows 32*(h%2)..+32, rest zero
        for h in range(H):
            nc.vector.memset(qT_pad[h][:], 0.0)
            ps = pp_big.tile([DH, RPC], F32, tag="ps")
            for ch in range(2):
                nc.tensor.matmul(
                    ps[:],
                    wq_sb[:, ch, h * DH:(h + 1) * DH],
                    xhatoT[:, ch, :],
                    start=(ch == 0),
                    stop=(ch == 1) and not has_qkv_bias,
                )
            if has_qkv_bias:
                nc.tensor.matmul(
                    ps[:], b_q[:, h * DH:(h + 1) * DH], onesrow_sb[:],
                    start=False, stop=True,
                )
            off = DH * (h % 2)
            nc.scalar.copy(out=qT_pad[h][off:off + DH, :], in_=ps[:])

        # ---------- Phase 4: scores -> exp -> mask -> att@v ----------
        # psum_y[:, rc, h, 0:32] accumulates y; [.., 32] the denominator.
        # One PSUM bank per rc (512 f32 = 2KB zero region): exactly one
        # accumulation group per bank — started by the first matmul that
        # touches it, stopped by the last.
        psum_y = pp_y.tile([P, OT, H, 64], F32)
        for j in range(NT):
            mt = mpool.tile([P, RPC], BF16, tag="mt")
            nc.sync.dma_start(out=mt[:], in_=maskT_d[j * P:(j + 1) * P, :])
            # same mask tile broadcast over the head-pair dim
            mt2 = bass.AP(
                tensor=mt.tensor, offset=mt.offset,
                ap=[mt.ap[0], [0, 2], mt.ap[1]],
            )
            for hp in range(H // 2):  # head pairs
                ps = pp_big.tile([P, 2, RPC], F32, tag="ps")
                for i in range(2):
                    nc.tensor.matmul(
                        ps[:, i, :],
                        kT[hp][:, j * P:(j + 1) * P],
                        qT_pad[2 * hp + i][:],
                        start=True, stop=True,
                    )
                at = apool.tile([P, 2, RPC], BF16, tag="at")
                nc.scalar.activation(out=at[:], in_=ps[:], func=AF.Exp)
                nc.vector.tensor_mul(at[:], at[:], mt2)
                for i in range(2):
                    h = 2 * hp + i
                    first = (j == 0) and (h == 0)
                    last = (j == NT - 1) and (h == H - 1)
                    for rc in range(OT):
                        nc.tensor.matmul(
                            psum_y[:, rc, h, 0:DH + 1],
                            at[:, i, rc * P:(rc + 1) * P],
                            v_aug[:, j, h, :],
                            start=first, stop=last,
                        )

        # ---------- Phase 5: normalize y ----------
        for h in range(H):
            for rc in range(OT):
                r = spool.tile([P, 1], F32, tag="recip")
                nc.vector.reciprocal(out=r[:], in_=psum_y[:, rc, h, DH:DH + 1])
                nc.vector.tensor_scalar(
                    out=y_sb[:, rc, h * DH:(h + 1) * DH],
                    in0=psum_y[:, rc, h, 0:DH],
                    scalar1=r[:], scalar2=None, op0=OP.mult,
                )

        # ---------- Phase 6: out-proj + residual + LN2 ----------
        nc.sync.dma_start(out=y_b.rearrange("(t p) d -> p t d", p=P), in_=y_sb[:])
        yT = persist.tile([P, 2, RPC], BF16)
        for half in range(2):
            nc.sync.dma_start_transpose(
                yT[:, half, :], y_b[:, half * P:(half + 1) * P]
            )
        for rc in range(OT):
            ps = pp_big.tile([P, D], F32, tag="ps")
            for ch in range(2):
                nc.tensor.matmul(
                    ps[:],
                    yT[:, ch, rc * P:(rc + 1) * P],
                    wo_sb[:, ch, :],
                    start=(ch == 0),
                    stop=(ch == 1) and not has_o_bias,
                )
            if has_o_bias:
                nc.tensor.matmul(
                    ps[:], onesrow_sb[:, rc * P:(rc + 1) * P], b_o[:],
                    start=False, stop=True,
                )
            nc.vector.tensor_add(x2_sb[:, rc, :], ps[:], x_own_sb[:, rc, :])
            # LN2 stats + normalize
            st = spool.tile([P, 6], F32, tag="st2")
            nc.vector.bn_stats(out=st[:], in_=x2_sb[:, rc, :])
            mv = spool.tile([P, 2], F32, tag="mv2")
            nc.vector.bn_aggr(out=mv[:], in_=st[:])
            sd = spool.tile([P, 1], F32, tag="sd2")
            nc.scalar.activation(
                out=sd[:], in_=mv[:, 1:2], func=AF.Sqrt, bias=eps_sb[:], scale=1.0
            )
            s = spool.tile([P, 1], F32, tag="s2")
            nc.vector.reciprocal(out=s[:], in_=sd[:])
            t = spool.tile([P, 1], F32, tag="t2")
            nc.vector.scalar_tensor_tensor(
                out=t[:], in0=mv[:, 0:1], scalar=-1.0, in1=s[:],
                op0=OP.mult, op1=OP.mult,
            )
            nc.vector.tensor_scalar(
                out=x2h_sb[:, rc, :], in0=x2_sb[:, rc, :], scalar1=s[:],
                scalar2=t[:], op0=OP.mult, op1=OP.add,
            )

        # ---------- Phase 7: MLP ----------
        nc.sync.dma_start(
            out=x2h_b.rearrange("(t p) d -> p t d", p=P), in_=x2h_sb[:]
        )
        x2hT = persist.tile([P, 2, RPC], BF16)
        for half in range(2):
            nc.sync.dma_start_transpose(
                x2hT[:, half, :], x2h_b[:, half * P:(half + 1) * P]
            )
        for rc in range(OT):
            ps = pp_big.tile([P, 2 * D], F32, tag="ps")
            for ch in range(2):
                nc.tensor.matmul(
                    ps[:],
                    x2hT[:, ch, rc * P:(rc + 1) * P],
                    wm1_sb[:, ch, :],
                    start=(ch == 0),
                    stop=(ch == 1) and not has_m1_bias,
                )
            if has_m1_bias:
                nc.tensor.matmul(
                    ps[:], onesrow_sb[:, rc * P:(rc + 1) * P], b_m1[:],
                    start=False, stop=True,
                )
            nc.scalar.activation(out=m1_sb[:, rc, :], in_=ps[:], func=AF.Gelu)
        nc.sync.dma_start(
            out=m1_b.rearrange("(t p) d -> p t d", p=P), in_=m1_sb[:]
        )
        for ch in range(4):
            nc.sync.dma_start_transpose(
                m1T[:, ch, :], m1_b[:, ch * P:(ch + 1) * P]
            )
        for rc in range(OT):
            ps = pp_big.tile([P, D], F32, tag="ps")
            for ch in range(4):
                nc.tensor.matmul(
                    ps[:],
                    m1T[:, ch, rc * P:(rc + 1) * P],
                    wm2_sb[:, ch, :],
                    start=(ch == 0),
                    stop=(ch == 3) and not has_m2_bias,
                )
            if has_m2_bias:
                nc.tensor.matmul(
                    ps[:], onesrow_sb[:, rc * P:(rc + 1) * P], b_m2[:],
                    start=False, stop=True,
                )
            nc.vector.tensor_add(out_sb[:, rc, :], ps[:], x2_sb[:, rc, :])
        nc.sync.dma_start(
            out=out_d.rearrange("(t p) d -> p t d", p=P), in_=out_sb[:]
        )

    nc.compile()
    return nc


def _bf16(a):
    return np.ascontiguousarray(a.astype(ml_dtypes.bfloat16))


def kernel(x, edge_index, Wq, bq, Wk, bk, Wv, bv, Wo, bo,
           g1, b1, g2, b2, Wm1, bm1, Wm2, bm2):
    x = np.asarray(x, np.float32)
    edge_index = np.asarray(edge_index)
    f32 = lambda a: np.asarray(a, np.float32)
    Wq, bq, Wk, bk, Wv, bv = map(f32, (Wq, bq, Wk, bk, Wv, bv))
    Wo, bo, g1, b1, g2, b2 = map(f32, (Wo, bo, g1, b1, g2, b2))
    Wm1, bm1, Wm2, bm2 = map(f32, (Wm1, bm1, Wm2, bm2))

    scale = 1.0 / math.sqrt(DH)

    # fold LN gains/shifts into the consuming weights
    Wq_f = (g1[:, None] * Wq) * scale
    bq_f = (b1 @ Wq + bq) * scale
    Wk_f = g1[:, None] * Wk
    bk_f = b1 @ Wk + bk
    Wv_f = g1[:, None] * Wv
    bv_f = b1 @ Wv + bv
    Wm1_f = g2[:, None] * Wm1
    bm1_f = b2 @ Wm1 + bm1

    has_qkv_bias = bool(np.any(bq_f) or np.any(bk_f) or np.any(bv_f))
    has_o_bias = bool(np.any(bo))
    has_m1_bias = bool(np.any(bm1_f))
    has_m2_bias = bool(np.any(bm2))

    key = (has_qkv_bias, has_o_bias, has_m1_bias, has_m2_bias)
    if key not in _CACHE:
        _CACHE[key] = _build(*key)
    nc = _CACHE[key]

    # mask (True where attended)
    src = np.asarray(edge_index[0], np.int64)
    dst = np.asarray(edge_index[1], np.int64)
    M = np.zeros((N, N), np.bool_)
    M[src, dst] = True

    wkv = np.concatenate([Wk_f, Wv_f], axis=1)  # [256, 512]
    wkv_u = _bf16(wkv.reshape(2, P, 2 * D))
    wq_u = _bf16(Wq_f.reshape(2, P, D))
    wo_u = _bf16(Wo.reshape(2, P, D))
    wm1_u = _bf16(Wm1_f.reshape(2, P, 2 * D))
    wm2_u = _bf16(Wm2.reshape(4, P, D))
    bias_u = _bf16(np.concatenate(
        [bk_f, bv_f, bq_f, bo, bm1_f, bm2]).reshape(1, 1792))

    in_maps = []
    for c in range(NCORES):
        rows = slice(c * RPC, (c + 1) * RPC)
        in_maps.append({
            "x_full": x,
            "x_own": np.ascontiguousarray(x[rows]),
            "maskT": _bf16(M[rows, :].T.astype(np.float32)),
            "wkv": wkv_u, "wq": wq_u, "wo": wo_u,
            "wm1": wm1_u, "wm2": wm2_u, "bias": bias_u,
        })

    global _last_in_maps
    _last_in_maps = in_maps
    res = run_bass_kernel_spmd(nc, in_maps, list(range(NCORES)))
    out = np.concatenate([res.results[c]["out"] for c in range(NCORES)], axis=0)
    return out.astype(np.float32)


_last_in_maps = None


if __name__ == "__main__":
    import reference
    inputs = {k: np.asarray(v) for k, v in reference.setup_inputs().items()}
    got = kernel(**inputs)
    exp = np.asarray(reference.reference(**reference.setup_inputs()))
    err = np.abs(got - exp)
    rel = np.abs(got - exp) / (np.abs(exp) + 1e-6)
    denom = np.maximum(np.abs(exp).max(), 1e-6)
    print("abs max err:", err.max(), "rel(scale):", err.max() / denom)
    print("mean rel:", (err / denom).mean())


# revision 30
# speedup vs baseline: 1.1343x; 1.1343x over previous
"""Trainium2 Bass kernel: sparse-attention transformer block.

Reference computation (N=4096, D=256, H=8, DH=32):
    h  = LN(x; g1, b1)
    q, k, v = (h@Wq+bq, h@Wk+bk, h@Wv+bv) split into 8 heads of 32
    att = softmax over edge-masked q k^T / sqrt(32)   (mask from edge_index)
    x  = x + att@v @ Wo + bo
    x  = x + gelu(LN(x; g2, b2) @ Wm1 + bm1) @ Wm2 + bm2

Sharding: rows are split 512 per core across 8 NeuronCores. Every matmul,
softmax row, layernorm row and residual is row-local, so there are no
collectives. Each core redundantly computes k/v (and LN1 stats) for all 4096
rows. LN gains/shifts are folded into the weight matrices on the host
(Wq <- diag(g1) Wq etc.), so the device only materializes the pure
normalization x_hat = (x - mean) * rsqrt(var + eps).

All transposes the PE needs are produced either by computing W^T @ x_hat^T
directly or via one DRAM-bounce DMA transpose (bf16).
"""

import math

import numpy as np
import ml_dtypes

import concourse.bass as bass
import concourse.bacc as bacc
import concourse.tile as tile
from concourse import mybir
from concourse.bass_utils import run_bass_kernel_spmd

N = 4096
D = 256
H = 8
DH = 32
NCORES = 8
RPC = N // NCORES  # rows per core = 512
P = 128
EPS = 1e-5
BF16 = mybir.dt.bfloat16
F32 = mybir.dt.float32

_CACHE = {}


def _build(has_qkv_bias, has_o_bias, has_m1_bias, has_m2_bias):
    nc = bacc.Bacc("TRN2", target_bir_lowering=False, debug=False)
    AF = mybir.ActivationFunctionType
    OP = mybir.AluOpType

    # ---------------- DRAM I/O ----------------
    x_full_d = nc.dram_tensor("x_full", [N, D], F32, kind="ExternalInput")
    x_own_d = nc.dram_tensor("x_own", [RPC, D], F32, kind="ExternalInput")
    maskT_d = nc.dram_tensor("maskT", [N, RPC], BF16, kind="ExternalInput")
    wkv_d = nc.dram_tensor("wkv", [2, P, 2 * D], BF16, kind="ExternalInput")
    wq_d = nc.dram_tensor("wq", [2, P, D], BF16, kind="ExternalInput")
    wo_d = nc.dram_tensor("wo", [2, P, D], BF16, kind="ExternalInput")
    wm1_d = nc.dram_tensor("wm1", [2, P, 2 * D], BF16, kind="ExternalInput")
    wm2_d = nc.dram_tensor("wm2", [4, P, D], BF16, kind="ExternalInput")
    # biases packed [kv(512) | q(256) | o(256) | m1(512) | m2(256)] = 1792
    bias_d = nc.dram_tensor("bias", [1, 1792], BF16, kind="ExternalInput")
    ident_d = nc.dram_tensor("ident", [P, P], BF16, kind="ExternalInput")
    out_d = nc.dram_tensor("out", [RPC, D], F32, kind="ExternalOutput")

    NT = N // P  # 32 full-row tiles
    OT = RPC // P  # 4 own-row tiles

    from contextlib import ExitStack
    with tile.TileContext(nc) as tc, ExitStack() as es:
        dram = es.enter_context(tc.tile_pool(name="dram", bufs=1, space="DRAM"))
        persist = es.enter_context(tc.tile_pool(name="persist", bufs=1))
        xpool = es.enter_context(tc.tile_pool(name="xpool", bufs=6))
        spool = es.enter_context(tc.tile_pool(name="spool", bufs=8))
        mpool = es.enter_context(tc.tile_pool(name="mpool", bufs=8))
        apool = es.enter_context(tc.tile_pool(name="apool", bufs=10))
        pp_big = es.enter_context(tc.tile_pool(name="pp_big", bufs=2, space="PSUM"))
        pp_y = es.enter_context(tc.tile_pool(name="pp_y", bufs=1, space="PSUM"))


        # persistent SBUF
        xhat_sb = persist.tile([P, NT, D], BF16)
        xhato_sb = persist.tile([P, OT, D], BF16)
        x_own_sb = persist.tile([P, OT, D], F32)
        xhatT = [persist.tile([P, N], BF16, name=f"xhatT{i}") for i in range(2)]
        xhatoT = persist.tile([P, 2, RPC], BF16)
        # k^T stored as head-pair tensors [64, N]; q^T per head zero-padded to
        # [64, RPC] so the scores matmul contracts 64 partitions from base 0.
        kT = [persist.tile([2 * DH, N], BF16, name=f"kT{i}") for i in range(4)]
        qT_pad = [persist.tile([2 * DH, RPC], BF16, name=f"qT{h}") for h in range(H)]
        v_aug = persist.tile([P, NT, H, DH + 1], BF16)
        y_sb = persist.tile([P, OT, D], BF16)
        x2_sb = persist.tile([P, OT, D], F32)
        x2h_sb = persist.tile([P, OT, D], BF16)
        m1_sb = persist.tile([P, OT, 2 * D], BF16)
        m1T = persist.tile([P, 4, RPC], BF16)
        out_sb = persist.tile([P, OT, D], F32)
        wkv_sb = persist.tile([P, 2, 2 * D], BF16)
        wq_sb = persist.tile([P, 2, D], BF16)
        wo_sb = persist.tile([P, 2, D], BF16)
        wm1_sb = persist.tile([P, 2, 2 * D], BF16)
        wm2_sb = persist.tile([P, 4, D], BF16)
        bias_sb = persist.tile([1, 1792], BF16)
        ones_sb = persist.tile([P, 1], BF16)
        onesrow_sb = persist.tile([1, RPC], BF16)
        eps_sb = persist.tile([P, 1], F32)
        ident_sb = persist.tile([P, P], BF16)

        nc.vector.memset(ones_sb[:], 1.0)
        nc.vector.memset(onesrow_sb[:], 1.0)
        nc.vector.memset(eps_sb[:], EPS)

        # weights in
        nc.sync.dma_start(out=wkv_sb[:], in_=wkv_d.rearrange("c p f -> p c f"))
        nc.sync.dma_start(out=wq_sb[:], in_=wq_d.rearrange("c p f -> p c f"))
        nc.sync.dma_start(out=wo_sb[:], in_=wo_d.rearrange("c p f -> p c f"))
        nc.sync.dma_start(out=wm1_sb[:], in_=wm1_d.rearrange("c p f -> p c f"))
        nc.sync.dma_start(out=wm2_sb[:], in_=wm2_d.rearrange("c p f -> p c f"))
        nc.sync.dma_start(out=bias_sb[:], in_=bias_d[:])
        nc.sync.dma_start(out=ident_sb[:], in_=ident_d[:])

        def pe_transpose(dst, src_sb, blocks):
            """dst[:, g, :] gets the PE-transposed [128,128] blocks; 4 blocks
            share one PSUM bank as a single accumulation group."""
            for g, blk4 in enumerate(blocks):
                ps = pp_big.tile([P, 4, P], BF16, tag="ps")
                for i, b in enumerate(blk4):
                    nc.tensor.matmul(
                        ps[:, i, :], b, ident_sb[:],
                        is_transpose=True,
                        start=(i == 0), stop=(i == len(blk4) - 1),
                    )
                nc.any.tensor_copy(dst[:, g, :], ps[:].rearrange("p a b -> p (a b)"))
        b_kv = bias_sb[:, 0:512]
        b_q = bias_sb[:, 512:768]
        b_o = bias_sb[:, 768:1024]
        b_m1 = bias_sb[:, 1024:1536]
        b_m2 = bias_sb[:, 1536:1792]

        # ---------- Phase 1: LN1 stats + x_hat (full, and own rows) ----------
        def ln_tiles(src_ap_fn, ntiles, dst_sb, keep_x=None):
            for i in range(ntiles):
                xt = xpool.tile([P, D], F32, tag="xt")
                nc.sync.dma_start(out=xt[:], in_=src_ap_fn(i))
                st = spool.tile([P, 6], F32, tag="st")
                nc.vector.bn_stats(out=st[:], in_=xt[:])
                mv = spool.tile([P, 2], F32, tag="mv")
                nc.vector.bn_aggr(out=mv[:], in_=st[:])
                sd = spool.tile([P, 1], F32, tag="sd")
                nc.scalar.activation(
                    out=sd[:], in_=mv[:, 1:2], func=AF.Sqrt, bias=eps_sb[:], scale=1.0
                )
                s = spool.tile([P, 1], F32, tag="s")
                nc.vector.reciprocal(out=s[:], in_=sd[:])
                t = spool.tile([P, 1], F32, tag="t")
                nc.vector.scalar_tensor_tensor(
                    out=t[:], in0=mv[:, 0:1], scalar=-1.0, in1=s[:],
                    op0=OP.mult, op1=OP.mult,
                )
                nc.vector.tensor_scalar(
                    out=dst_sb[:, i, :], in0=xt[:], scalar1=s[:], scalar2=t[:],
                    op0=OP.mult, op1=OP.add,
                )

        ln_tiles(lambda i: x_full_d[i * P:(i + 1) * P, :], NT, xhat_sb)
        ln_tiles(lambda i: x_own_d[i * P:(i + 1) * P, :], OT, xhato_sb)
        nc.sync.dma_start(
            out=x_own_sb[:], in_=x_own_d.rearrange("(t p) d -> p t d", p=P)
        )

        # ---------- Phase 2: bounce + transpose x_hat ----------
        for half in range(2):
            for g4 in range(NT // 4):
                ps = pp_big.tile([P, 4, P], BF16, tag="ps")
                for i in range(4):
                    t = 4 * g4 + i
                    nc.tensor.matmul(
                        ps[:, i, :],
                        xhat_sb[:, t, half * P:(half + 1) * P],
                        ident_sb[:], is_transpose=True,
                        start=(i == 0), stop=(i == 3),
                    )
                nc.any.tensor_copy(
                    xhatT[half][:, g4 * 4 * P:(g4 + 1) * 4 * P],
                    ps[:].rearrange("p a b -> p (a b)"),
                )
        pe_transpose(xhatoT, xhato_sb, [
            [xhato_sb[:, rc, half * P:(half + 1) * P] for rc in range(OT)]
            for half in range(2)
        ])

        # ---------- Phase 3: projections kT, v, qT ----------
        # kT[pair][do, c] = sum_di Wk[di, 64*pair + do] xhatT[di, c]
        G = 8  # 512-wide column groups of kT, two per PSUM tile
        for pair in range(4):
            for g2 in range(G // 2):
                ps = pp_big.tile([2 * DH, 2, 512], F32, tag="ps")
                for sub in range(2):
                    g = 2 * g2 + sub
                    cs = slice(g * 512, (g + 1) * 512)
                    for ch in range(2):
                        nc.tensor.matmul(
                            ps[:, sub, :],
                            wkv_sb[:, ch, pair * 64:(pair + 1) * 64],
                            xhatT[ch][:, cs],
                            start=(ch == 0),
                            stop=(ch == 1) and not has_qkv_bias,
                        )
                    if has_qkv_bias:
                        # += bk[do] * ones[c]
                        nc.tensor.matmul(
                            ps[:, sub, :],
                            b_kv[:, pair * 64:(pair + 1) * 64],
                            onesrow_sb[:, 0:512],
                            start=False, stop=True,
                        )
                nc.any.tensor_copy(
                    kT[pair][:, g2 * 1024:(g2 + 1) * 1024],
                    ps[:].rearrange("p a b -> p (a b)"),
                )
        # v[r, dv] tiles, copied into v_aug [P, j, h, 33]: 32 head dims plus a
        # constant-1 column so att@v and the softmax denominator share matmuls
        nc.vector.memset(v_aug[:], 1.0)
        for i2 in range(NT // 2):
            ps = pp_big.tile([P, 2, D], F32, tag="ps")
            for sub in range(2):
                i = 2 * i2 + sub
                for ch in range(2):
                    nc.tensor.matmul(
                        ps[:, sub, :],
                        xhatT[ch][:, i * P:(i + 1) * P],
                        wkv_sb[:, ch, D:2 * D],
                        start=(ch == 0),
                        stop=(ch == 1) and not has_qkv_bias,
                    )
                if has_qkv_bias:
                    nc.tensor.matmul(
                        ps[:, sub, :], onesrow_sb[:, 0:P], b_kv[:, D:2 * D],
                        start=False, stop=True,
                    )
            nc.any.tensor_copy(
                v_aug[:, 2 * i2:2 * i2 + 2, :, 0:DH],
                ps[:].rearrange("p s (h d) -> p s h d", h=H),
            )
        # qT_pad[h][do, r own]: head h occupies rows 32*(h%2)..+32, rest zero
        for h in range(H):
            nc.vector.memset(qT_pad[h][:], 0.0)
            ps = pp_big.tile([DH, RPC], F32, tag="ps")
            for ch in range(2):
                nc.tensor.matmul(
                    ps[:],
                    wq_sb[:, ch, h * DH:(h + 1) * DH],
                    xhatoT[:, ch, :],
                    start=(ch == 0),
                    stop=(ch == 1) and not has_qkv_bias,
                )
            if has_qkv_bias:
                nc.tensor.matmul(
                    ps[:], b_q[:, h * DH:(h + 1) * DH], onesrow_sb[:],
                    start=False, stop=True,
                )
            off = DH * (h % 2)
            nc.scalar.copy(out=qT_pad[h][off:off + DH, :], in_=ps[:])

        # ---------- Phase 4: scores -> exp -> mask -> att@v ----------
        # psum_y[:, rc, h, 0:32] accumulates y; [.., 32] the denominator.
        # One PSUM bank per rc (512 f32 = 2KB zero region): exactly one
        # accumulation group per bank — started by the first matmul that
        # touches it, stopped by the last.
        psum_y = pp_y.tile([P, OT, H, 64], F32)
        for j in range(NT):
            mt = mpool.tile([P, RPC], BF16, tag="mt")
            nc.sync.dma_start(out=mt[:], in_=maskT_d[j * P:(j + 1) * P, :])
            # same mask tile broadcast over the head-pair dim
            mt2 = bass.AP(
                tensor=mt.tensor, offset=mt.offset,
                ap=[mt.ap[0], [0, 2], mt.ap[1]],
            )
            for hp in range(H // 2):  # head pairs
                ps = pp_big.tile([P, 2, RPC], F32, tag="ps")
                for i in range(2):
                    nc.tensor.matmul(
                        ps[:, i, :],
                        kT[hp][:, j * P:(j + 1) * P],
                        qT_pad[2 * hp + i][:],
                        start=True, stop=True,
                    )
                at = apool.tile([P, 2, RPC], BF16, tag="at")
                nc.scalar.activation(out=at[:], in_=ps[:], func=AF.Exp)
                nc.vector.tensor_mul(at[:], at[:], mt2)
                for i in range(2):
                    h = 2 * hp + i
                    first = (j == 0) and (h == 0)
                    last = (j == NT - 1) and (h == H - 1)
                    for rc in range(OT):
                        nc.tensor.matmul(
                            psum_y[:, rc, h, 0:DH + 1],
                            at[:, i, rc * P:(rc + 1) * P],
                            v_aug[:, j, h, :],
                            start=first, stop=last,
                        )

        # ---------- Phase 5: normalize y ----------
        for h in range(H):
            for rc in range(OT):
                r = spool.tile([P, 1], F32, tag="recip")
                nc.vector.reciprocal(out=r[:], in_=psum_y[:, rc, h, DH:DH + 1])
                nc.vector.tensor_scalar(
                    out=y_sb[:, rc, h * DH:(h + 1) * DH],
                    in0=psum_y[:, rc, h, 0:DH],
                    scalar1=r[:], scalar2=None, op0=OP.mult,
                )

        # ---------- Phase 6: out-proj + residual + LN2 ----------
        yT = persist.tile([P, 2, RPC], BF16)
        pe_transpose(yT, y_sb, [
            [y_sb[:, rc, half * P:(half + 1) * P] for rc in range(OT)]
            for half in range(2)
        ])
        for rc in range(OT):
            ps = pp_big.tile([P, D], F32, tag="ps")
            for ch in range(2):
                nc.tensor.matmul(
                    ps[:],
                    yT[:, ch, rc * P:(rc + 1) * P],
                    wo_sb[:, ch, :],
                    start=(ch == 0),
                    stop=(ch == 1) and not has_o_bias,
                )
            if has_o_bias:
                nc.tensor.matmul(
                    ps[:], onesrow_sb[:, rc * P:(rc + 1) * P], b_o[:],
                    start=False, stop=True,
                )
            nc.vector.tensor_add(x2_sb[:, rc, :], ps[:], x_own_sb[:, rc, :])
            # LN2 stats + normalize
            st = spool.tile([P, 6], F32, tag="st2")
            nc.vector.bn_stats(out=st[:], in_=x2_sb[:, rc, :])
            mv = spool.tile([P, 2], F32, tag="mv2")
            nc.vector.bn_aggr(out=mv[:], in_=st[:])
            sd = spool.tile([P, 1], F32, tag="sd2")
            nc.scalar.activation(
                out=sd[:], in_=mv[:, 1:2], func=AF.Sqrt, bias=eps_sb[:], scale=1.0
            )
            s = spool.tile([P, 1], F32, tag="s2")
            nc.vector.reciprocal(out=s[:], in_=sd[:])
            t = spool.tile([P, 1], F32, tag="t2")
            nc.vector.scalar_tensor_tensor(
                out=t[:], in0=mv[:, 0:1], scalar=-1.0, in1=s[:],
                op0=OP.mult, op1=OP.mult,
            )
            nc.vector.tensor_scalar(
                out=x2h_sb[:, rc, :], in0=x2_sb[:, rc, :], scalar1=s[:],
                scalar2=t[:], op0=OP.mult, op1=OP.add,
            )

        # ---------- Phase 7: MLP ----------
        x2hT = persist.tile([P, 2, RPC], BF16)
        pe_transpose(x2hT, x2h_sb, [
            [x2h_sb[:, rc, half * P:(half + 1) * P] for rc in range(OT)]
            for half in range(2)
        ])
        for rc in range(OT):
            ps = pp_big.tile([P, 2 * D], F32, tag="ps")
            for ch in range(2):
                nc.tensor.matmul(
                    ps[:],
                    x2hT[:, ch, rc * P:(rc + 1) * P],
                    wm1_sb[:, ch, :],
                    start=(ch == 0),
                    stop=(ch == 1) and not has_m1_bias,
                )
            if has_m1_bias:
                nc.tensor.matmul(
                    ps[:], onesrow_sb[:, rc * P:(rc + 1) * P], b_m1[:],
                    start=False, stop=True,
                )
            nc.scalar.activation(out=m1_sb[:, rc, :], in_=ps[:], func=AF.Gelu)
        pe_transpose(m1T, m1_sb, [
            [m1_sb[:, rc, ch * P:(ch + 1) * P] for rc in range(OT)]
            for ch in range(4)
        ])
        for rc in range(OT):
            ps = pp_big.tile([P, D], F32, tag="ps")
            for ch in range(4):
                nc.tensor.matmul(
                    ps[:],
                    m1T[:, ch, rc * P:(rc + 1) * P],
                    wm2_sb[:, ch, :],
                    start=(ch == 0),
                    stop=(ch == 3) and not has_m2_bias,
                )
            if has_m2_bias:
                nc.tensor.matmul(
                    ps[:], onesrow_sb[:, rc * P:(rc + 1) * P], b_m2[:],
                    start=False, stop=True,
                )
            nc.vector.tensor_add(out_sb[:, rc, :], ps[:], x2_sb[:, rc, :])
        nc.sync.dma_start(
            out=out_d.rearrange("(t p) d -> p t d", p=P), in_=out_sb[:]
        )

    nc.compile()
    return nc


def _bf16(a):
    return np.ascontiguousarray(a.astype(ml_dtypes.bfloat16))


def kernel(x, edge_index, Wq, bq, Wk, bk, Wv, bv, Wo, bo,
           g1, b1, g2, b2, Wm1, bm1, Wm2, bm2):
    x = np.asarray(x, np.float32)
    edge_index = np.asarray(edge_index)
    f32 = lambda a: np.asarray(a, np.float32)
    Wq, bq, Wk, bk, Wv, bv = map(f32, (Wq, bq, Wk, bk, Wv, bv))
    Wo, bo, g1, b1, g2, b2 = map(f32, (Wo, bo, g1, b1, g2, b2))
    Wm1, bm1, Wm2, bm2 = map(f32, (Wm1, bm1, Wm2, bm2))

    scale = 1.0 / math.sqrt(DH)

    # fold LN gains/shifts into the consuming weights
    Wq_f = (g1[:, None] * Wq) * scale
    bq_f = (b1 @ Wq + bq) * scale
    Wk_f = g1[:, None] * Wk
    bk_f = b1 @ Wk + bk
    Wv_f = g1[:, None] * Wv
    bv_f = b1 @ Wv + bv
    Wm1_f = g2[:, None] * Wm1
    bm1_f = b2 @ Wm1 + bm1

    has_qkv_bias = bool(np.any(bq_f) or np.any(bk_f) or np.any(bv_f))
    has_o_bias = bool(np.any(bo))
    has_m1_bias = bool(np.any(bm1_f))
    has_m2_bias = bool(np.any(bm2))

    key = (has_qkv_bias, has_o_bias, has_m1_bias, has_m2_bias)
    if key not in _CACHE:
        _CACHE[key] = _build(*key)
    nc = _CACHE[key]

    # mask (True where attended)
    src = np.asarray(edge_index[0], np.int64)
    dst = np.asarray(edge_index[1], np.int64)
    M = np.zeros((N, N), np.bool_)
    M[src, dst] = True

    wkv = np.concatenate([Wk_f, Wv_f], axis=1)  # [256, 512]
    wkv_u = _bf16(wkv.reshape(2, P, 2 * D))
    wq_u = _bf16(Wq_f.reshape(2, P, D))
    wo_u = _bf16(Wo.reshape(2, P, D))
    wm1_u = _bf16(Wm1_f.reshape(2, P, 2 * D))
    wm2_u = _bf16(Wm2.reshape(4, P, D))
    bias_u = _bf16(np.concatenate(
        [bk_f, bv_f, bq_f, bo, bm1_f, bm2]).reshape(1, 1792))
    ident_u = _bf16(np.eye(P, dtype=np.float32))

    in_maps = []
    for c in range(NCORES):
        rows = slice(c * RPC, (c + 1) * RPC)
        in_maps.append({
            "x_full": x,
            "x_own": np.ascontiguousarray(x[rows]),
            "maskT": _bf16(M[rows, :].T.astype(np.float32)),
            "wkv": wkv_u, "wq": wq_u, "wo": wo_u,
            "wm1": wm1_u, "wm2": wm2_u, "bias": bias_u, "ident": ident_u,
        })

    global _last_in_maps
    _last_in_maps = in_maps
    res = run_bass_kernel_spmd(nc, in_maps, list(range(NCORES)))
    out = np.concatenate([res.results[c]["out"] for c in range(NCORES)], axis=0)
    return out.astype(np.float32)


_last_in_maps = None


if __name__ == "__main__":
    import reference
    inputs = {k: np.asarray(v) for k, v in reference.setup_inputs().items()}
    got = kernel(**inputs)
    exp = np.asarray(reference.reference(**reference.setup_inputs()))
    err = np.abs(got - exp)
    rel = np.abs(got - exp) / (np.abs(exp) + 1e-6)
    denom = np.maximum(np.abs(exp).max(), 1e-6)
    print("abs max err:", err.max(), "rel(scale):", err.max() / denom)
    print("mean rel:", (err / denom).mean())
